# revision 65
# baseline (speedup 1.0000x reference)
"""Trainium2 Bass kernel for the cached-transformer-encoder-layer problem.

Strategy (8 NeuronCores, SPMD, zero collectives):
  - Shard the B*S = 6144 token rows across 8 cores (768 rows each); cores
    0-3 take batch 0, cores 4-7 take batch 1.  Each core runs the full
    layer for its tokens.  Softmax attention is permutation-invariant in
    the keys, so K/V are laid out [cached | recomputed] and never
    scattered.
  - Matmuls run in fp8e4 with perf_mode=DoubleRow wherever precision
    allows: projections fuse contraction-chunk PAIRS into single
    0.5-cycle/row instructions (4x over bf16); attention scores use a
    stride-0 broadcast k/q pair (computes 2*K^T Q, the 2 is folded into
    the softmax scale, 2x); attention ctx contracts key-chunk pairs of
    fp8 probabilities (4x).  The FFN stays bf16: its quantization error
    hits the output un-damped, unlike attention whose contribution is
    ~100x smaller than the residual stream.
  - Softmax exp is the single biggest engine cost (147K columns, a hard
    ~123us floor on ACT alone).  It is split ~4:3 between ACT (native
    Exp -> fp8) and DVE (Schraudolph bit-trick exp: uint8 bits =
    a*score + b, bitcast to fp8e4 -- the e4m3 format is piecewise-linear
    in log2).  gpsimd cannot read PSUM, so it takes the SBUF-side
    elementwise work (LN applies, copies, broadcasts) instead.  The
    per-query normalizer Z comes free as a ones-column baked into V.
  - LayerNorm statistics via ones-vector matmuls on the PE; rstd =
    exp(-0.5*ln(var+eps)) so ACT stays pinned to the single activation
    table (natural_log_exp_and_others) holding Exp/Ln/Relu/Identity --
    an explicit LoadActFuncSet up front stops Bacc's greedy inserter
    from thrashing 1.3us table loads between Exp and Ln tables.
  - The attention inner loop is software-pipelined one quad ahead (ctx
    matmuls emitted one quad behind scores) so the PE writes quad g+1
    while the exp engines work on quad g; three pipelined token slices
    overlap attention with the previous slice's out-proj/LN/FFN tail.
  - Fast path is specialized to the problem spec's zero out-proj/FFN/LN
    biases and unit LN gains (input_specs fills); a host-side guard
    falls back to an exact numpy port for any other inputs.

kernel(**inputs) takes the FULL unsharded inputs and returns the FULL
[B, S, D] output; host numpy does the slicing / transposes / fp8
quantization and the final assembly.
"""

import numpy as np

B, S, D, H, DFF = 2, 3072, 512, 8, 2048
HD = D // H              # 64
R = 768                  # recomputed tokens
SC = S - R               # 2304 cached tokens
EPS = 1e-5
P = 128
N_CORES = 8
Q = (B * S) // N_CORES   # 768 query rows per core
DC = D // P              # 4 chunks of the model dim
FC = DFF // P            # 16 chunks of the FFN dim
KC = S // P              # 24 key chunks
CC = SC // P             # 18 cached key chunks
NSPLIT = ((0, 512), (512, 768))      # moving-dim splits (PSUM bank=512 f32)
SLICES = ((0, 256), (256, 512), (512, 768))
_CACHE = {}

# Schraudolph exp: e4m3 bit pattern ~ 8*(log2(v)+7); score psum carries an
# extra factor 16 (2 from the stride-0 DoubleRow trick, 8 from 1/sqrt(HD)).
_SCHA = 8.0 * float(np.log2(np.e)) / 16.0
_SCHB = 56.0 + 0.5 - 0.32            # +0.5 trunc->round, -0.32 sawtooth ctr

# exp engine schedule per quad (cycled): A=ACT native, D=DVE Schraudolph
# (gpsimd cannot read PSUM, so it takes SBUF-side work instead of exp)
EXP_SCHED = "AADADAD"


def _build_program():
    import concourse.bacc as bacc
    import concourse.mybir as mybir
    import concourse.tile as tile

    import bass_rust
    from concourse.hw_specs import get_activation_tables

    f32 = mybir.dt.float32
    f32r = mybir.dt.float32r
    bf16 = mybir.dt.bfloat16
    fp8 = mybir.dt.float8e4
    u8 = mybir.dt.uint8
    AF = mybir.ActivationFunctionType
    OP = mybir.AluOpType
    PM = mybir.MatmulPerfMode

    nc = bacc.Bacc("TRN2", target_bir_lowering=False, debug=False,
                   num_devices=N_CORES)

    # ---- DRAM I/O (partition-major host layouts) ---------------------
    d_src = nc.dram_tensor("srcP", [P, DC * Q], f32r, kind="ExternalInput")
    d_src8 = nc.dram_tensor("src8P", [P, DC * Q], fp8, kind="ExternalInput")
    d_srcR8 = nc.dram_tensor("srcR8P", [P, DC * R], fp8, kind="ExternalInput")
    d_wq8 = nc.dram_tensor("wq8P", [P, 2 * 2 * D], fp8, kind="ExternalInput")
    d_wk8 = nc.dram_tensor("wk8P", [P, 2 * 2 * D], fp8, kind="ExternalInput")
    d_wv8 = nc.dram_tensor("wv8P", [P, 2 * 2 * D], fp8, kind="ExternalInput")
    d_wo8 = nc.dram_tensor("wo8P", [P, 2 * 2 * D], fp8, kind="ExternalInput")
    d_w1 = nc.dram_tensor("w1P", [P, DC * DFF], bf16, kind="ExternalInput")
    d_w2 = nc.dram_tensor("w2P", [P, FC * D], bf16, kind="ExternalInput")
    d_kc8 = nc.dram_tensor("kc8P", [P, 4 * SC], fp8, kind="ExternalInput")
    # V pair layout padded to stride 80 (DoubleRow k-tile step must be %16)
    VP = 80
    d_vv8 = nc.dram_tensor("vv8P", [P, H * 9 * 2 * VP], fp8,
                           kind="ExternalInput")
    d_vecs = nc.dram_tensor("vecsP", [P, DC * 9], f32, kind="ExternalInput")
    d_bvrow = nc.dram_tensor("bvrow", [P, D], f32, kind="ExternalInput")
    d_ones = nc.dram_tensor("onesc", [P, 1], f32r, kind="ExternalInput")
    # bias ROWS for the ones-row bias-matmul trick (bias folded into the
    # PSUM accumulation so post-matmul ops batch across chunk pairs)
    d_b1c = nc.dram_tensor("b1c", [P, FC], f32, kind="ExternalInput")
    d_outs = [nc.dram_tensor(f"out{k}", [P, DC * 256], f32r,
                             kind="ExternalOutput") for k in range(3)]

    def rr(ap, cols):  # [P, n*cols] -> [P, n, cols]
        return ap.rearrange("p (o q) -> p o q", q=cols)

    exp_n = [0]

    with tile.TileContext(nc) as tc:
        with (
            tc.tile_pool(name="sb", bufs=1) as sb,
            tc.tile_pool(name="hp", bufs=2) as hp,
            tc.tile_pool(name="sqp", bufs=2) as sqp,
            tc.tile_pool(name="prp", bufs=6) as prp,
            tc.tile_pool(name="zbp", bufs=2) as zbp,
            tc.tile_pool(name="stp", bufs=2) as stp,
            tc.tile_pool(name="ps_s", bufs=2, space="PSUM") as ps_s,
            tc.tile_pool(name="ps_ctx", bufs=1, space="PSUM") as ps_ctx,
            tc.tile_pool(name="ps_b", bufs=3, space="PSUM") as ps_b,
        ):
            # ---- phase 0: loads, critical-path first -----------------
            sb_wq8 = sb.tile([P, 2, 2, D], fp8, tag="wq8")
            sb_wk8 = sb.tile([P, 2, 2, D], fp8, tag="wk8")
            sb_wv8 = sb.tile([P, 2, 2, D], fp8, tag="wv8")
            s8 = sb.tile([P, DC, Q], fp8, tag="s8")
            sR8 = sb.tile([P, DC, R], fp8, tag="sR8")
            kc8 = sb.tile([P, 4, SC], fp8, tag="kc8")
            vv8 = sb.tile([P, H, KC // 2, 2, VP], fp8, tag="vv8")
            vvr = d_vv8.ap().rearrange("p (h c t d) -> p h c t d",
                                       h=H, c=9, t=2)
            # critical path first: what head 0's first quads need
            nc.sync.dma_start(sb_wq8[:], rr(d_wq8.ap(), D))
            nc.gpsimd.dma_start(s8[:], rr(d_src8.ap(), Q))
            nc.sync.dma_start(kc8[:, 0], rr(d_kc8.ap(), SC)[:, 0])
            nc.sync.dma_start(vv8[:, 0:2, 0:9], vvr[:, 0:2])
            nc.sync.dma_start(sb_wk8[:], rr(d_wk8.ap(), D))
            nc.gpsimd.dma_start(sR8[:], rr(d_srcR8.ap(), R))
            nc.sync.dma_start(sb_wv8[:], rr(d_wv8.ap(), D))
            nc.sync.dma_start(kc8[:, 1], rr(d_kc8.ap(), SC)[:, 1])
            nc.sync.dma_start(vv8[:, 2:4, 0:9], vvr[:, 2:4])
            nc.sync.dma_start(kc8[:, 2], rr(d_kc8.ap(), SC)[:, 2])
            nc.sync.dma_start(vv8[:, 4:6, 0:9], vvr[:, 4:6])
            nc.sync.dma_start(kc8[:, 3], rr(d_kc8.ap(), SC)[:, 3])
            nc.sync.dma_start(vv8[:, 6:8, 0:9], vvr[:, 6:8])
            sb_vecs = sb.tile([P, DC, 9], f32, tag="vecs")
            nc.gpsimd.dma_start(sb_vecs[:], rr(d_vecs.ap(), 9))
            sb_bv = sb.tile([P, D], f32, tag="bv")
            nc.gpsimd.dma_start(sb_bv[:], d_bvrow.ap())
            ones_col = sb.tile([P, 1], f32r, tag="ones")
            nc.gpsimd.dma_start(ones_col[:], d_ones.ap())
            sb_b1 = sb.tile([P, FC], f32, tag="b1")
            nc.gpsimd.dma_start(sb_b1[:], d_b1c.ap())
            sb_src = sb.tile([P, DC, Q], f32r, tag="src")
            for o in range(DC):
                nc.gpsimd.dma_start(sb_src[:, o], rr(d_src.ap(), Q)[:, o])
            sb_eps = sb.tile([1, 1], f32, tag="eps")
            nc.gpsimd.memset(sb_eps[:], EPS)

            # Pin the ACT engine to the one table holding every function
            # this kernel uses (Exp/Ln/Relu/Identity); the automatic
            # inserter would otherwise thrash Exp<->Ln tables (1.3us each).
            tabs = list(get_activation_tables(nc.m.arch).items())
            tid = [i for i, (n, _) in enumerate(tabs)
                   if n == "natural_log_exp_and_others"][0]
            nc.scalar.add_instruction(bass_rust.InstLoadActFuncSet(
                name=nc.get_next_instruction_name(), ins=[], outs=[],
                act_func_set_id=tid))

            # (no PE warm-up: the early schedule is exp-paced with little
            # PE work, so the clock-gate ramp costs less than the ~5us a
            # warm-up burn would delay the first q-proj/scores)

            def col(o, j):  # per-partition scalar column j, chunk o of vecs
                return sb_vecs[:, o, j:j + 1]

            # ---- phase 1: projections (fp8 DoubleRow, T layout) ------
            q8 = sb.tile([P, DC, Q], fp8, tag="q8")
            kr8 = sb.tile([P, DC, R], fp8, tag="kr8")
            vr8 = sb.tile([P, KC - CC, D], fp8, tag="vr8")

            def qk_pieces(m):
                ps = []
                for c0, c1 in NSPLIT:
                    def pq(m=m, c0=c0, c1=c1):
                        pqp = ps_b.tile([P, c1 - c0], f32, tag="b")
                        for j in range(2):
                            nc.tensor.matmul(
                                pqp[:], sb_wq8[:, j, :, P * m:P * (m + 1)],
                                s8[:, 2 * j:2 * j + 2, c0:c1],
                                start=(j == 0), stop=(j == 1),
                                perf_mode=PM.DoubleRow)
                        nc.vector.tensor_scalar(
                            out=q8[:, m, c0:c1], in0=pqp[:],
                            scalar1=col(m, 0), scalar2=None, op0=OP.add)
                    ps.append(pq)
                for c0, c1 in NSPLIT:
                    def pk(m=m, c0=c0, c1=c1):
                        pkp = ps_b.tile([P, c1 - c0], f32, tag="b")
                        for j in range(2):
                            nc.tensor.matmul(
                                pkp[:], sb_wk8[:, j, :, P * m:P * (m + 1)],
                                sR8[:, 2 * j:2 * j + 2, c0:c1],
                                start=(j == 0), stop=(j == 1),
                                perf_mode=PM.DoubleRow)
                        nc.vector.tensor_scalar(
                            out=kr8[:, m, c0:c1], in0=pkp[:],
                            scalar1=col(m, 1), scalar2=None, op0=OP.add)
                    ps.append(pk)
                return ps

            def qk_proj(m):
                for p in qk_pieces(m):
                    p()

            def v_proj():
                for vg in range(2):
                    for t in range(KC - CC):
                        pv = ps_b.tile([P, 256], f32, tag="b")
                        for j in range(2):
                            nc.tensor.matmul(
                                pv[:], sR8[:, 2 * j:2 * j + 2,
                                           P * t:P * (t + 1)],
                                sb_wv8[:, j, :, 256 * vg:256 * (vg + 1)],
                                start=(j == 0), stop=(j == 1),
                                perf_mode=PM.DoubleRow)
                        nc.vector.tensor_tensor(
                            out=vr8[:, t, 256 * vg:256 * (vg + 1)],
                            in0=pv[:],
                            in1=sb_bv[:, 256 * vg:256 * (vg + 1)], op=OP.add)
                # scatter recomputed V into the per-head pair layout
                for h in range(H):
                    nc.gpsimd.tensor_copy(
                        out=vv8[:, h, 9:12, :, 0:HD],
                        in_=vr8[:, :, HD * h:HD * (h + 1)].rearrange(
                            "p (a b) d -> p a b d", b=2))
                nc.gpsimd.memset(vv8[:, :, 9:12, :, HD:HD + 1], 1.0)
                nc.gpsimd.memset(vv8[:, :, 9:12, :, HD + 1:], 0.0)

            qk_proj(0)
            v_proj()

            sb_wo8 = sb.tile([P, 2, 2, D], fp8, tag="wo8")
            nc.sync.dma_start(sb_wo8[:], rr(d_wo8.ap(), D))
            sb_w1 = sb.tile([P, DC, DFF], bf16, tag="w1")
            nc.sync.dma_start(sb_w1[:], rr(d_w1.ap(), DFF))
            sb_w2 = sb.tile([P, FC, D], bf16, tag="w2")
            nc.sync.dma_start(sb_w2[:], rr(d_w2.ap(), D))

            h16 = sb.tile([P, FC, Q], bf16, tag="h16")
            ctxh_t = {}
            pctx_t = {}

            # PE filler: tail/projection work paced into the attention
            # stream so the PE's idle slots (while ACT/DVE chew exp) are
            # used instead of piling the FFN up after attention ends.
            # ---- attention quad: 4 score chunks -> exp -> 2 ctx pairs.
            # ctx is emitted one quad late (software pipeline) so the PE
            # writes quad g+1 while the exp engines work on quad g.
            def attn_scores(sl, h, g):
                t0, t1 = SLICES[sl]
                W = t1 - t0
                hr = HD * (h % 2)
                i = h // 2
                if g == 0:
                    pctx_t[(sl, h)] = ps_ctx.tile([HD + 1, W], f32,
                                                  tag="ctx",
                                                  name=f"pctx_{sl}_{h}")
                psq = ps_s.tile([P, 4, W], f32, tag="s")
                for u in range(4):
                    c = 4 * g + u
                    if c < CC:
                        lhs = kc8[hr:hr + HD, i, P * c:P * (c + 1)]
                    else:
                        cc = c - CC
                        lhs = kr8[hr:hr + HD, i, P * cc:P * (cc + 1)]
                    nc.tensor.matmul(
                        psq[:, u, :],
                        lhs.unsqueeze(1).broadcast_to([HD, 2, P]),
                        q8[hr:hr + HD, i, t0:t1].unsqueeze(1)
                          .broadcast_to([HD, 2, W]),
                        start=(u % 2 == 0), stop=(u % 2 == 1),
                        perf_mode=PM.DoubleRow)
                eng = EXP_SCHED[exp_n[0] % len(EXP_SCHED)]
                exp_n[0] += 1
                pru = prp.tile([P, 4, W], u8, tag="pr")
                if eng == "A":
                    nc.scalar.activation(
                        out=pru[:].bitcast(fp8), in_=psq[:], func=AF.Exp,
                        scale=1.0 / 16.0)
                else:
                    e = nc.vector if eng == "D" else nc.gpsimd
                    e.tensor_scalar(
                        out=pru[:], in0=psq[:], scalar1=_SCHA,
                        scalar2=_SCHB, op0=OP.mult, op1=OP.add)
                return (sl, h, g, pru)

            def attn_ctx(sl, h, g, pru):
                t0, t1 = SLICES[sl]
                W = t1 - t0
                pctx = pctx_t[(sl, h)]
                for half in range(2):
                    cp = 2 * g + half
                    nc.tensor.matmul(
                        pctx[:, 0:W],
                        vv8[:, h, cp, :, 0:HD + 1],
                        pru[:, 2 * half:2 * half + 2, :].bitcast(fp8),
                        start=(cp == 0), stop=(cp == KC // 2 - 1),
                        perf_mode=PM.DoubleRow)
                if g == KC // 4 - 1:
                    # head done: normalize by the ones-column Z
                    hr = HD * (h % 2)
                    i = h // 2
                    zi = zbp.tile([1, W], f32, tag="zi")
                    nc.vector.reciprocal(zi[:], pctx[HD:HD + 1, 0:W])
                    zb = zbp.tile([HD, W], f32, tag="zb")
                    nc.gpsimd.partition_broadcast(zb[:], zi[:])
                    nc.vector.tensor_tensor(
                        out=ctxh_t[t0][hr:hr + HD, i, 0:W],
                        in0=pctx[0:HD, 0:W], in1=zb[:], op=OP.mult)

            # ---- LayerNorm over the feature dim (partitions) ---------
            def _ln_cols(xt, gj, bj, W, xq=None, fast=False):
                ee = nc.vector if fast else nc.gpsimd
                psum = ps_b.tile([1, W], f32, tag="b")
                psq = ps_b.tile([1, W], f32, tag="b")
                for o in range(DC):
                    sq = sqp.tile([P, W], f32r, tag="sq")
                    ee.tensor_mul(sq[:], xt[:, o, 0:W], xt[:, o, 0:W])
                    nc.tensor.matmul(
                        psum[0:1, 0:W], ones_col[:], xt[:, o, 0:W],
                        start=(o == 0), stop=(o == DC - 1))
                    nc.tensor.matmul(
                        psq[0:1, 0:W], ones_col[:], sq[:],
                        start=(o == 0), stop=(o == DC - 1))
                # m = psum/D; u = m^2; v = psq/D - u; rstd = (v+eps)^-0.5
                # (each op reads at most one PSUM operand)
                st = stp.tile([1, 4 * W], f32, tag="st")
                m, u = st[0:1, 2 * W:3 * W], st[0:1, 3 * W:]
                rstd, mrs = st[0:1, 0:W], st[0:1, W:2 * W]
                nc.vector.tensor_scalar_mul(m, psum[0:1, :], 1.0 / D)
                ee.tensor_tensor(out=u, in0=m, in1=m, op=OP.mult)
                nc.vector.scalar_tensor_tensor(
                    out=u, in0=psq[0:1, :], scalar=1.0 / D, in1=u,
                    op0=OP.mult, op1=OP.subtract)
                # rstd = exp(-0.5*ln(var+eps)); Ln+Exp share an ACT table
                nc.scalar.activation(out=rstd, in_=u, func=AF.Ln,
                                     bias=sb_eps[:])
                nc.scalar.activation(out=rstd, in_=rstd, func=AF.Exp,
                                     scale=-0.5)
                ee.tensor_tensor(out=mrs, in0=m, in1=rstd,
                                  op=OP.mult)
                rb = stp.tile([P, 2 * W], f32, tag="rb")
                nc.gpsimd.partition_broadcast(rb[:], st[0:1, 0:2 * W])
                rstd_b = rb[:, 0:W]
                mrs_b = rb[:, W:]
                for o in range(DC):
                    ee.tensor_tensor(
                        out=xt[:, o, 0:W], in0=xt[:, o, 0:W],
                        in1=rstd_b, op=OP.mult)
                    ee.tensor_tensor(
                        out=xt[:, o, 0:W], in0=xt[:, o, 0:W],
                        in1=mrs_b, op=OP.subtract)
                    if xq is not None:
                        nc.gpsimd.tensor_copy(
                            out=xq[:, o, 0:W], in_=xt[:, o, 0:W])

            def tail_a(t0, t1, sbase=None):
                W = t1 - t0
                sbase = t0 if sbase is None else sbase
                lc0 = t0 - sbase
                ctxh = ctxh_t[sbase]
                xsb = hp.tile([P, DC, W], f32r, tag="xh", name=f"xh_{t0}")
                x16 = hp.tile([P, DC, W], bf16, tag="x16h",
                              name=f"x16h_{t0}")
                for m in range(DC):
                    pa = ps_b.tile([P, W], f32, tag="b")
                    for j in range(2):
                        nc.tensor.matmul(
                            pa[:], sb_wo8[:, j, :, P * m:P * (m + 1)],
                            ctxh[:, 2 * j:2 * j + 2, lc0:lc0 + W],
                            start=(j == 0), stop=(j == 1),
                            perf_mode=PM.DoubleRow)
                    nc.vector.tensor_tensor(
                        out=xsb[:, m, 0:W], in0=pa[:],
                        in1=sb_src[:, m, t0:t1], op=OP.add)
                _ln_cols(xsb, 5, 6, W, xq=x16)
                return xsb, x16

            def tail_b(t0, t1, xsb, x16, sbase=None, fast=False):
                W = t1 - t0
                sbase = t0 if sbase is None else sbase
                lc0 = t0 - sbase
                for f in range(FC):
                    ph = ps_b.tile([P, W], f32, tag="b")
                    for o in range(DC):
                        nc.tensor.matmul(
                            ph[:], sb_w1[:, o, P * f:P * (f + 1)],
                            x16[:, o, 0:W],
                            start=(o == 0), stop=(o == DC - 1))
                    nc.scalar.activation(
                        out=h16[:, f, t0:t1], in_=ph[:], func=AF.Relu)
                ysb = hp.tile([P, DC, W], f32r, tag="yh", name=f"yh_{t0}")
                for m in range(DC):
                    py = ps_b.tile([P, W], f32, tag="b")
                    for f in range(FC):
                        nc.tensor.matmul(
                            py[:], sb_w2[:, f, P * m:P * (m + 1)],
                            h16[:, f, t0:t1],
                            start=(f == 0), stop=(f == FC - 1))
                    nc.vector.tensor_tensor(
                        out=ysb[:, m, 0:W], in0=py[:],
                        in1=xsb[:, m, 0:W], op=OP.add)
                _ln_cols(ysb, 7, 8, W, fast=fast)
                oap = rr(d_outs[sbase // 256].ap(), 256)
                for o in range(DC):   # per-chunk: store overlaps normalize
                    nc.sync.dma_start(oap[:, o, lc0:lc0 + W],
                                      ysb[:, o, 0:W])

            def attn_head(sl, h):
                pend = None
                for g in range(KC // 4):
                    new = attn_scores(sl, h, g)
                    if pend is not None:
                        attn_ctx(*pend)
                    pend = new
                attn_ctx(*pend)

            def attn_slice(sl):
                for h in range(H):
                    attn_head(sl, h)

            # ---- pipelined emission order ----------------------------
            for sl in range(3):
                t0, t1 = SLICES[sl]
                ctxh_t[t0] = hp.tile([P, DC, t1 - t0], fp8, tag="ctxh",
                                     name=f"ctxh_{t0}")
            attn_head(0, 0)
            attn_head(0, 1)
            qk_proj(1)
            attn_head(0, 2)
            attn_head(0, 3)
            qk_proj(2)
            attn_head(0, 4)
            attn_head(0, 5)
            qk_proj(3)
            attn_head(0, 6)
            attn_head(0, 7)
            attn_slice(1)
            x0 = tail_a(*SLICES[0])
            attn_slice(2)
            tail_b(*SLICES[0], *x0)
            x1 = tail_a(*SLICES[1])
            # the final slice's tail heads the drain critical path: give
            # it emission priority over slice 1's bulk FFN, which can fill
            # the drain's engine gaps instead
            x2 = tail_a(*SLICES[2])
            tail_b(*SLICES[1], *x1)
            tail_b(*SLICES[2], *x2)

    nc.compile()
    return nc


def _get_program():
    if "nc" not in _CACHE:
        _CACHE["nc"] = _build_program()
    return _CACHE["nc"]


def _numpy_reference(src, recompute_idx, cached_idx, k_cached, v_cached,
                     in_proj_w, in_proj_b, out_proj_w, out_proj_b,
                     w1, b1, w2, b2, norm1_w, norm1_b, norm2_w, norm2_b):
    """Exact numpy translation of the oracle (general-case fallback)."""
    f = np.float32
    src = np.asarray(src, f)
    wq, wk, wv = in_proj_w[:D], in_proj_w[D:2 * D], in_proj_w[2 * D:]
    bq, bk, bv = in_proj_b[:D], in_proj_b[D:2 * D], in_proj_b[2 * D:]

    def ln(x, g, b):
        m = x.mean(-1, keepdims=True)
        v = x.var(-1, keepdims=True)
        return (x - m) / np.sqrt(v + EPS) * g + b

    q = (src @ wq.T + bq).reshape(B, S, H, HD).transpose(0, 2, 1, 3)
    src_rec = src[:, recompute_idx, :]
    k_rec = (src_rec @ wk.T + bk).reshape(B, -1, H, HD).transpose(0, 2, 1, 3)
    v_rec = (src_rec @ wv.T + bv).reshape(B, -1, H, HD).transpose(0, 2, 1, 3)
    k_full = np.zeros((B, H, S, HD), f)
    v_full = np.zeros((B, H, S, HD), f)
    k_full[:, :, cached_idx, :] = np.asarray(k_cached, f)[None]
    v_full[:, :, cached_idx, :] = np.asarray(v_cached, f)[None]
    k_full[:, :, recompute_idx, :] = k_rec
    v_full[:, :, recompute_idx, :] = v_rec
    scale = f(1.0 / np.sqrt(HD))
    scores = np.einsum("bhqd,bhkd->bhqk", q, k_full).astype(f) * scale
    scores -= scores.max(-1, keepdims=True)
    e = np.exp(scores)
    attn = e / e.sum(-1, keepdims=True)
    ctx = np.einsum("bhqk,bhkd->bhqd", attn, v_full).astype(f)
    ctx = ctx.transpose(0, 2, 1, 3).reshape(B, S, D)
    attn_out = ctx @ out_proj_w.T + out_proj_b
    x = ln(src + attn_out, norm1_w, norm1_b)
    ffn = np.maximum(x @ w1.T + b1, 0.0) @ w2.T + b2
    return ln(x + ffn, norm2_w, norm2_b).astype(f)


def _fp8(a):
    import ml_dtypes
    return np.ascontiguousarray(a).astype(ml_dtypes.float8_e4m3fn)


def _bf16(a):
    import ml_dtypes
    return np.ascontiguousarray(a).astype(ml_dtypes.bfloat16)


def _pmaj(x):
    """[n*P, cols] -> partition-major [P, n*cols] (contiguous)."""
    n = x.shape[0] // P
    return np.ascontiguousarray(
        x.reshape(n, P, x.shape[1]).transpose(1, 0, 2).reshape(P, -1))


def _pack_dr(wT, npair):
    """[npair*2*128, cols] -> DoubleRow layout [P, npair*2*cols]."""
    cols = wT.shape[1]
    return _fp8(wT.reshape(npair, 2, P, cols).transpose(2, 0, 1, 3)
                .reshape(P, -1))


def kernel(**inputs) -> np.ndarray:
    f = np.float32
    src = np.ascontiguousarray(np.asarray(inputs["src"], f))
    ridx = np.asarray(inputs["recompute_idx"]).astype(np.int64)
    cidx = np.asarray(inputs["cached_idx"]).astype(np.int64)

    # The fast path relies on {cached_idx} + {recompute_idx} being a
    # disjoint partition of [0, S), and is specialized to the problem
    # spec's zero FFN/out-proj/LN biases and unit LN gains (input_specs
    # fills).  Anything else -> exact numpy fallback.
    allidx = np.concatenate([ridx, cidx])
    if (len(ridx) != R or len(cidx) != SC
            or not np.array_equal(np.sort(allidx), np.arange(S))
            or np.any(np.asarray(inputs["out_proj_b"]))
            or np.any(np.asarray(inputs["b1"]))
            or np.any(np.asarray(inputs["b2"]))
            or np.any(np.asarray(inputs["norm1_b"]))
            or np.any(np.asarray(inputs["norm2_b"]))
            or not np.all(np.asarray(inputs["norm1_w"]) == 1.0)
            or not np.all(np.asarray(inputs["norm2_w"]) == 1.0)):
        return _numpy_reference(**inputs)

    in_proj_w = np.asarray(inputs["in_proj_w"], f)
    in_proj_b = np.asarray(inputs["in_proj_b"], f)
    out_proj_w = np.asarray(inputs["out_proj_w"], f)
    out_proj_b = np.asarray(inputs["out_proj_b"], f)
    w1 = np.asarray(inputs["w1"], f)
    b1 = np.asarray(inputs["b1"], f)
    w2 = np.asarray(inputs["w2"], f)
    b2 = np.asarray(inputs["b2"], f)
    k_cached = np.asarray(inputs["k_cached"], f)
    v_cached = np.asarray(inputs["v_cached"], f)

    wq, wk, wv = in_proj_w[:D], in_proj_w[D:2 * D], in_proj_w[2 * D:]
    bq, bk, bv = in_proj_b[:D], in_proj_b[D:2 * D], in_proj_b[2 * D:]

    wq8 = _pack_dr(np.ascontiguousarray(wq.T), 2)
    wk8 = _pack_dr(np.ascontiguousarray(wk.T), 2)
    wv8 = _pack_dr(np.ascontiguousarray(wv.T), 2)
    wo8 = _pack_dr(np.ascontiguousarray(out_proj_w.T), 2)
    w1P = _bf16(_pmaj(np.ascontiguousarray(w1.T)))
    w2P = _bf16(_pmaj(np.ascontiguousarray(w2.T)))
    vecsP = _pmaj(np.ascontiguousarray(np.stack(
        [bq, bk, bv, out_proj_b, b2,
         np.asarray(inputs["norm1_w"], f), np.asarray(inputs["norm1_b"], f),
         np.asarray(inputs["norm2_w"], f), np.asarray(inputs["norm2_b"], f)],
        axis=1)))
    bvrow = np.ascontiguousarray(np.tile(bv[None, :], (P, 1)))
    # packed cached-K: kc8[64*(h%2)+d, h//2, s] = k_cached[h, s, d]
    kct = k_cached.transpose(0, 2, 1)                  # [H, HD, SC]
    kc8 = _fp8(kct.reshape(4, 2, HD, SC).transpose(1, 2, 0, 3)
               .reshape(P, 4 * SC))
    # cached V in chunk-pair layout: ones column + zero pad to stride 80
    vca = np.concatenate(
        [v_cached.reshape(H, CC, P, HD), np.ones((H, CC, P, 1), f),
         np.zeros((H, CC, P, 80 - HD - 1), f)], axis=3)
    vv8 = _fp8(vca.transpose(2, 0, 1, 3).reshape(P, -1))

    shared = {
        "wq8P": wq8, "wk8P": wk8, "wv8P": wv8, "wo8P": wo8,
        "w1P": w1P, "w2P": w2P, "kc8P": kc8, "vv8P": vv8,
        "vecsP": vecsP, "bvrow": bvrow,
        "onesc": np.ones((P, 1), f),
        "b1c": np.ascontiguousarray(b1.reshape(FC, P).T),
    }
    srcR8 = [_fp8(_pmaj(np.ascontiguousarray(src[b][ridx].T)))
             for b in range(B)]

    in_maps = []
    for c in range(N_CORES):
        b, t = divmod(c, N_CORES // B)
        m = dict(shared)
        sP = _pmaj(np.ascontiguousarray(src[b, Q * t:Q * (t + 1), :].T))
        m["srcP"] = sP
        m["src8P"] = _fp8(sP)
        m["srcR8P"] = srcR8[b]
        in_maps.append(m)

    from concourse import bass_utils
    nc = _get_program()
    res = bass_utils.run_bass_kernel_spmd(
        nc, in_maps, core_ids=list(range(N_CORES)))

    out = np.empty((B, S, D), f)
    for c in range(N_CORES):
        b, t = divmod(c, N_CORES // B)
        outP = np.concatenate(
            [res.results[c][f"out{k}"].reshape(P, DC, 256)
             for k in range(3)], axis=2)        # [P, DC, Q]
        outT = outP.transpose(1, 0, 2).reshape(D, Q)
        out[b, Q * t:Q * (t + 1), :] = outT.T
    return out


# revision 67
# speedup vs baseline: 1.0307x; 1.0307x over previous
"""Trainium2 Bass kernel for the cached-transformer-encoder-layer problem.

Strategy (8 NeuronCores, SPMD, zero collectives):
  - Shard the B*S = 6144 token rows across 8 cores (768 rows each); cores
    0-3 take batch 0, cores 4-7 take batch 1.  Each core runs the full
    layer for its tokens.  Softmax attention is permutation-invariant in
    the keys, so K/V are laid out [cached | recomputed] and never
    scattered.
  - Matmuls run in fp8e4 with perf_mode=DoubleRow wherever precision
    allows: projections fuse contraction-chunk PAIRS into single
    0.5-cycle/row instructions (4x over bf16); attention scores use a
    stride-0 broadcast k/q pair (computes 2*K^T Q, the 2 is folded into
    the softmax scale, 2x); attention ctx contracts key-chunk pairs of
    fp8 probabilities (4x).  The FFN stays bf16: its quantization error
    hits the output un-damped, unlike attention whose contribution is
    ~100x smaller than the residual stream.
  - Softmax exp is the single biggest engine cost (147K columns, a hard
    ~123us floor on ACT alone).  It is split ~4:3 between ACT (native
    Exp -> fp8) and DVE (Schraudolph bit-trick exp: uint8 bits =
    a*score + b, bitcast to fp8e4 -- the e4m3 format is piecewise-linear
    in log2).  gpsimd cannot read PSUM, so it takes the SBUF-side
    elementwise work (LN applies, copies, broadcasts) instead.  The
    per-query normalizer Z comes free as a ones-column baked into V.
  - LayerNorm statistics via ones-vector matmuls on the PE; rstd =
    exp(-0.5*ln(var+eps)) so ACT stays pinned to the single activation
    table (natural_log_exp_and_others) holding Exp/Ln/Relu/Identity --
    an explicit LoadActFuncSet up front stops Bacc's greedy inserter
    from thrashing 1.3us table loads between Exp and Ln tables.
  - The attention inner loop is software-pipelined one quad ahead (ctx
    matmuls emitted one quad behind scores) so the PE writes quad g+1
    while the exp engines work on quad g; three pipelined token slices
    overlap attention with the previous slice's out-proj/LN/FFN tail.
  - Fast path is specialized to the problem spec's zero out-proj/FFN/LN
    biases and unit LN gains (input_specs fills); a host-side guard
    falls back to an exact numpy port for any other inputs.

kernel(**inputs) takes the FULL unsharded inputs and returns the FULL
[B, S, D] output; host numpy does the slicing / transposes / fp8
quantization and the final assembly.
"""

import numpy as np

B, S, D, H, DFF = 2, 3072, 512, 8, 2048
HD = D // H              # 64
R = 768                  # recomputed tokens
SC = S - R               # 2304 cached tokens
EPS = 1e-5
P = 128
N_CORES = 8
Q = (B * S) // N_CORES   # 768 query rows per core
DC = D // P              # 4 chunks of the model dim
FC = DFF // P            # 16 chunks of the FFN dim
KC = S // P              # 24 key chunks
CC = SC // P             # 18 cached key chunks
NSPLIT = ((0, 512), (512, 768))      # moving-dim splits (PSUM bank=512 f32)
SLICES = ((0, 256), (256, 512), (512, 768))
_CACHE = {}

# Schraudolph exp: e4m3 bit pattern ~ 8*(log2(v)+7); score psum carries an
# extra factor 16 (2 from the stride-0 DoubleRow trick, 8 from 1/sqrt(HD)).
_SCHA = 8.0 * float(np.log2(np.e)) / 16.0
_SCHB = 56.0 + 0.5 - 0.32            # +0.5 trunc->round, -0.32 sawtooth ctr

# exp engine schedule per quad (cycled): A=ACT native, D=DVE Schraudolph
# (gpsimd cannot read PSUM, so it takes SBUF-side work instead of exp)
EXP_SCHED = "AADADAD"


def _build_program():
    import concourse.bacc as bacc
    import concourse.mybir as mybir
    import concourse.tile as tile

    import bass_rust
    from concourse.hw_specs import get_activation_tables

    f32 = mybir.dt.float32
    f32r = mybir.dt.float32r
    bf16 = mybir.dt.bfloat16
    fp8 = mybir.dt.float8e4
    u8 = mybir.dt.uint8
    AF = mybir.ActivationFunctionType
    OP = mybir.AluOpType
    PM = mybir.MatmulPerfMode

    nc = bacc.Bacc("TRN2", target_bir_lowering=False, debug=False,
                   num_devices=N_CORES)

    # ---- DRAM I/O (partition-major host layouts) ---------------------
    d_src = nc.dram_tensor("srcP", [P, DC * Q], f32r, kind="ExternalInput")
    d_src8 = nc.dram_tensor("src8P", [P, DC * Q], fp8, kind="ExternalInput")
    d_srcR8 = nc.dram_tensor("srcR8P", [P, DC * R], fp8, kind="ExternalInput")
    d_wq8 = nc.dram_tensor("wq8P", [P, 2 * 2 * D], fp8, kind="ExternalInput")
    d_wk8 = nc.dram_tensor("wk8P", [P, 2 * 2 * D], fp8, kind="ExternalInput")
    d_wv8 = nc.dram_tensor("wv8P", [P, 2 * 2 * D], fp8, kind="ExternalInput")
    d_wo8 = nc.dram_tensor("wo8P", [P, 2 * 2 * D], fp8, kind="ExternalInput")
    d_w1 = nc.dram_tensor("w1P", [P, DC * DFF], bf16, kind="ExternalInput")
    d_w2 = nc.dram_tensor("w2P", [P, FC * D], bf16, kind="ExternalInput")
    d_kc8 = nc.dram_tensor("kc8P", [P, 4 * SC], fp8, kind="ExternalInput")
    # V pair layout padded to stride 80 (DoubleRow k-tile step must be %16)
    VP = 80
    d_vv8 = nc.dram_tensor("vv8P", [P, H * 9 * 2 * VP], fp8,
                           kind="ExternalInput")
    d_vecs = nc.dram_tensor("vecsP", [P, DC * 9], f32, kind="ExternalInput")
    d_bvrow = nc.dram_tensor("bvrow", [P, D], f32, kind="ExternalInput")
    d_ones = nc.dram_tensor("onesc", [P, 1], f32r, kind="ExternalInput")
    # bias ROWS for the ones-row bias-matmul trick (bias folded into the
    # PSUM accumulation so post-matmul ops batch across chunk pairs)
    d_b1c = nc.dram_tensor("b1c", [P, FC], f32, kind="ExternalInput")
    d_outs = [nc.dram_tensor(f"out{k}", [P, DC * 256], f32r,
                             kind="ExternalOutput") for k in range(3)]

    def rr(ap, cols):  # [P, n*cols] -> [P, n, cols]
        return ap.rearrange("p (o q) -> p o q", q=cols)

    exp_n = [0]

    with tile.TileContext(nc) as tc:
        with (
            tc.tile_pool(name="sb", bufs=1) as sb,
            tc.tile_pool(name="hp", bufs=2) as hp,
            tc.tile_pool(name="sqp", bufs=2) as sqp,
            tc.tile_pool(name="prp", bufs=6) as prp,
            tc.tile_pool(name="zbp", bufs=2) as zbp,
            tc.tile_pool(name="stp", bufs=2) as stp,
            tc.tile_pool(name="ps_s", bufs=2, space="PSUM") as ps_s,
            tc.tile_pool(name="ps_ctx", bufs=1, space="PSUM") as ps_ctx,
            tc.tile_pool(name="ps_b", bufs=3, space="PSUM") as ps_b,
        ):
            # ---- phase 0: loads, critical-path first -----------------
            sb_wq8 = sb.tile([P, 2, 2, D], fp8, tag="wq8")
            sb_wk8 = sb.tile([P, 2, 2, D], fp8, tag="wk8")
            sb_wv8 = sb.tile([P, 2, 2, D], fp8, tag="wv8")
            s8 = sb.tile([P, DC, Q], fp8, tag="s8")
            sR8 = sb.tile([P, DC, R], fp8, tag="sR8")
            kc8 = sb.tile([P, 4, SC], fp8, tag="kc8")
            vv8 = sb.tile([P, H, KC // 2, 2, VP], fp8, tag="vv8")
            vvr = d_vv8.ap().rearrange("p (h c t d) -> p h c t d",
                                       h=H, c=9, t=2)
            # critical path first: what head 0's first quads need
            nc.sync.dma_start(sb_wq8[:], rr(d_wq8.ap(), D))
            nc.gpsimd.dma_start(s8[:], rr(d_src8.ap(), Q))
            nc.sync.dma_start(kc8[:, 0:2], rr(d_kc8.ap(), SC)[:, 0:2])
            nc.sync.dma_start(vv8[:, 0:4, 0:9], vvr[:, 0:4])
            nc.sync.dma_start(sb_wk8[:], rr(d_wk8.ap(), D))
            nc.gpsimd.dma_start(sR8[:], rr(d_srcR8.ap(), R))
            nc.sync.dma_start(sb_wv8[:], rr(d_wv8.ap(), D))
            nc.sync.dma_start(kc8[:, 2:4], rr(d_kc8.ap(), SC)[:, 2:4])
            nc.sync.dma_start(vv8[:, 4:8, 0:9], vvr[:, 4:8])
            sb_vecs = sb.tile([P, DC, 9], f32, tag="vecs")
            nc.gpsimd.dma_start(sb_vecs[:], rr(d_vecs.ap(), 9))
            sb_bv = sb.tile([P, D], f32, tag="bv")
            nc.gpsimd.dma_start(sb_bv[:], d_bvrow.ap())
            ones_col = sb.tile([P, 1], f32r, tag="ones")
            nc.gpsimd.dma_start(ones_col[:], d_ones.ap())
            sb_b1 = sb.tile([P, FC], f32, tag="b1")
            nc.gpsimd.dma_start(sb_b1[:], d_b1c.ap())
            sb_src = sb.tile([P, DC, Q], f32r, tag="src")
            for o in range(DC):
                nc.gpsimd.dma_start(sb_src[:, o], rr(d_src.ap(), Q)[:, o])
            sb_eps = sb.tile([1, 1], f32, tag="eps")
            nc.gpsimd.memset(sb_eps[:], EPS)

            # Pin the ACT engine to the one table holding every function
            # this kernel uses (Exp/Ln/Relu/Identity); the automatic
            # inserter would otherwise thrash Exp<->Ln tables (1.3us each).
            tabs = list(get_activation_tables(nc.m.arch).items())
            tid = [i for i, (n, _) in enumerate(tabs)
                   if n == "natural_log_exp_and_others"][0]
            nc.scalar.add_instruction(bass_rust.InstLoadActFuncSet(
                name=nc.get_next_instruction_name(), ins=[], outs=[],
                act_func_set_id=tid))

            # (no PE warm-up: the early schedule is exp-paced with little
            # PE work, so the clock-gate ramp costs less than the ~5us a
            # warm-up burn would delay the first q-proj/scores)

            def col(o, j):  # per-partition scalar column j, chunk o of vecs
                return sb_vecs[:, o, j:j + 1]

            # ---- phase 1: projections (fp8 DoubleRow, T layout) ------
            q8 = sb.tile([P, DC, Q], fp8, tag="q8")
            kr8 = sb.tile([P, DC, R], fp8, tag="kr8")
            vr8 = sb.tile([P, KC - CC, D], fp8, tag="vr8")

            def qk_pieces(m):
                ps = []
                for c0, c1 in NSPLIT:
                    def pq(m=m, c0=c0, c1=c1):
                        pqp = ps_b.tile([P, c1 - c0], f32, tag="b")
                        for j in range(2):
                            nc.tensor.matmul(
                                pqp[:], sb_wq8[:, j, :, P * m:P * (m + 1)],
                                s8[:, 2 * j:2 * j + 2, c0:c1],
                                start=(j == 0), stop=(j == 1),
                                perf_mode=PM.DoubleRow)
                        nc.vector.tensor_scalar(
                            out=q8[:, m, c0:c1], in0=pqp[:],
                            scalar1=col(m, 0), scalar2=None, op0=OP.add)
                    ps.append(pq)
                for c0, c1 in NSPLIT:
                    def pk(m=m, c0=c0, c1=c1):
                        pkp = ps_b.tile([P, c1 - c0], f32, tag="b")
                        for j in range(2):
                            nc.tensor.matmul(
                                pkp[:], sb_wk8[:, j, :, P * m:P * (m + 1)],
                                sR8[:, 2 * j:2 * j + 2, c0:c1],
                                start=(j == 0), stop=(j == 1),
                                perf_mode=PM.DoubleRow)
                        nc.vector.tensor_scalar(
                            out=kr8[:, m, c0:c1], in0=pkp[:],
                            scalar1=col(m, 1), scalar2=None, op0=OP.add)
                    ps.append(pk)
                return ps

            def qk_proj(m):
                for p in qk_pieces(m):
                    p()

            def v_proj():
                for vg in range(2):
                    for t in range(KC - CC):
                        pv = ps_b.tile([P, 256], f32, tag="b")
                        for j in range(2):
                            nc.tensor.matmul(
                                pv[:], sR8[:, 2 * j:2 * j + 2,
                                           P * t:P * (t + 1)],
                                sb_wv8[:, j, :, 256 * vg:256 * (vg + 1)],
                                start=(j == 0), stop=(j == 1),
                                perf_mode=PM.DoubleRow)
                        nc.vector.tensor_tensor(
                            out=vr8[:, t, 256 * vg:256 * (vg + 1)],
                            in0=pv[:],
                            in1=sb_bv[:, 256 * vg:256 * (vg + 1)], op=OP.add)
                # scatter recomputed V into the per-head pair layout
                for h in range(H):
                    nc.gpsimd.tensor_copy(
                        out=vv8[:, h, 9:12, :, 0:HD],
                        in_=vr8[:, :, HD * h:HD * (h + 1)].rearrange(
                            "p (a b) d -> p a b d", b=2))
                nc.gpsimd.memset(vv8[:, :, 9:12, :, HD:HD + 1], 1.0)
                nc.gpsimd.memset(vv8[:, :, 9:12, :, HD + 1:], 0.0)

            qk_proj(0)
            v_proj()

            sb_wo8 = sb.tile([P, 2, 2, D], fp8, tag="wo8")
            nc.sync.dma_start(sb_wo8[:], rr(d_wo8.ap(), D))
            sb_w1 = sb.tile([P, DC, DFF], bf16, tag="w1")
            nc.sync.dma_start(sb_w1[:], rr(d_w1.ap(), DFF))
            sb_w2 = sb.tile([P, FC, D], bf16, tag="w2")
            nc.sync.dma_start(sb_w2[:], rr(d_w2.ap(), D))

            h16 = sb.tile([P, FC, Q], bf16, tag="h16")
            ctxh_t = {}
            pctx_t = {}

            # PE filler: tail/projection work paced into the attention
            # stream so the PE's idle slots (while ACT/DVE chew exp) are
            # used instead of piling the FFN up after attention ends.
            # ---- attention quad: 4 score chunks -> exp -> 2 ctx pairs.
            # ctx is emitted one quad late (software pipeline) so the PE
            # writes quad g+1 while the exp engines work on quad g.
            def attn_scores(sl, h, g):
                t0, t1 = SLICES[sl]
                W = t1 - t0
                hr = HD * (h % 2)
                i = h // 2
                if g == 0:
                    pctx_t[(sl, h)] = ps_ctx.tile([HD + 1, W], f32,
                                                  tag="ctx",
                                                  name=f"pctx_{sl}_{h}")
                psq = ps_s.tile([P, 4, W], f32, tag="s")
                for u in range(4):
                    c = 4 * g + u
                    if c < CC:
                        lhs = kc8[hr:hr + HD, i, P * c:P * (c + 1)]
                    else:
                        cc = c - CC
                        lhs = kr8[hr:hr + HD, i, P * cc:P * (cc + 1)]
                    nc.tensor.matmul(
                        psq[:, u, :],
                        lhs.unsqueeze(1).broadcast_to([HD, 2, P]),
                        q8[hr:hr + HD, i, t0:t1].unsqueeze(1)
                          .broadcast_to([HD, 2, W]),
                        start=(u % 2 == 0), stop=(u % 2 == 1),
                        perf_mode=PM.DoubleRow)
                eng = EXP_SCHED[exp_n[0] % len(EXP_SCHED)]
                exp_n[0] += 1
                pru = prp.tile([P, 4, W], u8, tag="pr")
                if eng == "A":
                    nc.scalar.activation(
                        out=pru[:].bitcast(fp8), in_=psq[:], func=AF.Exp,
                        scale=1.0 / 16.0)
                else:
                    e = nc.vector if eng == "D" else nc.gpsimd
                    e.tensor_scalar(
                        out=pru[:], in0=psq[:], scalar1=_SCHA,
                        scalar2=_SCHB, op0=OP.mult, op1=OP.add)
                return (sl, h, g, pru)

            def attn_ctx(sl, h, g, pru):
                t0, t1 = SLICES[sl]
                W = t1 - t0
                pctx = pctx_t[(sl, h)]
                for half in range(2):
                    cp = 2 * g + half
                    nc.tensor.matmul(
                        pctx[:, 0:W],
                        vv8[:, h, cp, :, 0:HD + 1],
                        pru[:, 2 * half:2 * half + 2, :].bitcast(fp8),
                        start=(cp == 0), stop=(cp == KC // 2 - 1),
                        perf_mode=PM.DoubleRow)
                if g == KC // 4 - 1:
                    # head done: normalize by the ones-column Z
                    hr = HD * (h % 2)
                    i = h // 2
                    zi = zbp.tile([1, W], f32, tag="zi")
                    nc.vector.reciprocal(zi[:], pctx[HD:HD + 1, 0:W])
                    zb = zbp.tile([HD, W], f32, tag="zb")
                    nc.gpsimd.partition_broadcast(zb[:], zi[:])
                    nc.vector.tensor_tensor(
                        out=ctxh_t[t0][hr:hr + HD, i, 0:W],
                        in0=pctx[0:HD, 0:W], in1=zb[:], op=OP.mult)

            # ---- LayerNorm over the feature dim (partitions) ---------
            def _ln_cols(xt, gj, bj, W, xq=None, fast=False):
                ee = nc.vector if fast else nc.gpsimd
                psum = ps_b.tile([1, W], f32, tag="b")
                psq = ps_b.tile([1, W], f32, tag="b")
                for o in range(DC):
                    sq = sqp.tile([P, W], f32r, tag="sq")
                    ee.tensor_mul(sq[:], xt[:, o, 0:W], xt[:, o, 0:W])
                    nc.tensor.matmul(
                        psum[0:1, 0:W], ones_col[:], xt[:, o, 0:W],
                        start=(o == 0), stop=(o == DC - 1))
                    nc.tensor.matmul(
                        psq[0:1, 0:W], ones_col[:], sq[:],
                        start=(o == 0), stop=(o == DC - 1))
                # m = psum/D; u = m^2; v = psq/D - u; rstd = (v+eps)^-0.5
                # (each op reads at most one PSUM operand)
                st = stp.tile([1, 4 * W], f32, tag="st")
                m, u = st[0:1, 2 * W:3 * W], st[0:1, 3 * W:]
                rstd, mrs = st[0:1, 0:W], st[0:1, W:2 * W]
                nc.vector.tensor_scalar_mul(m, psum[0:1, :], 1.0 / D)
                ee.tensor_tensor(out=u, in0=m, in1=m, op=OP.mult)
                nc.vector.scalar_tensor_tensor(
                    out=u, in0=psq[0:1, :], scalar=1.0 / D, in1=u,
                    op0=OP.mult, op1=OP.subtract)
                # rstd = exp(-0.5*ln(var+eps)); Ln+Exp share an ACT table
                nc.scalar.activation(out=rstd, in_=u, func=AF.Ln,
                                     bias=sb_eps[:])
                nc.scalar.activation(out=rstd, in_=rstd, func=AF.Exp,
                                     scale=-0.5)
                ee.tensor_tensor(out=mrs, in0=m, in1=rstd,
                                  op=OP.mult)
                rb = stp.tile([P, 2 * W], f32, tag="rb")
                nc.gpsimd.partition_broadcast(rb[:], st[0:1, 0:2 * W])
                rstd_b = rb[:, 0:W]
                mrs_b = rb[:, W:]
                for o in range(DC):
                    ee.tensor_tensor(
                        out=xt[:, o, 0:W], in0=xt[:, o, 0:W],
                        in1=rstd_b, op=OP.mult)
                    ee.tensor_tensor(
                        out=xt[:, o, 0:W], in0=xt[:, o, 0:W],
                        in1=mrs_b, op=OP.subtract)
                    if xq is not None:
                        nc.gpsimd.tensor_copy(
                            out=xq[:, o, 0:W], in_=xt[:, o, 0:W])

            def tail_a(t0, t1, sbase=None):
                W = t1 - t0
                sbase = t0 if sbase is None else sbase
                lc0 = t0 - sbase
                ctxh = ctxh_t[sbase]
                xsb = hp.tile([P, DC, W], f32r, tag="xh", name=f"xh_{t0}")
                x16 = hp.tile([P, DC, W], bf16, tag="x16h",
                              name=f"x16h_{t0}")
                for m in range(DC):
                    pa = ps_b.tile([P, W], f32, tag="b")
                    for j in range(2):
                        nc.tensor.matmul(
                            pa[:], sb_wo8[:, j, :, P * m:P * (m + 1)],
                            ctxh[:, 2 * j:2 * j + 2, lc0:lc0 + W],
                            start=(j == 0), stop=(j == 1),
                            perf_mode=PM.DoubleRow)
                    nc.vector.tensor_tensor(
                        out=xsb[:, m, 0:W], in0=pa[:],
                        in1=sb_src[:, m, t0:t1], op=OP.add)
                _ln_cols(xsb, 5, 6, W, xq=x16)
                return xsb, x16

            def tail_b(t0, t1, xsb, x16, sbase=None, fast=False):
                W = t1 - t0
                sbase = t0 if sbase is None else sbase
                lc0 = t0 - sbase
                for f in range(FC):
                    ph = ps_b.tile([P, W], f32, tag="b")
                    for o in range(DC):
                        nc.tensor.matmul(
                            ph[:], sb_w1[:, o, P * f:P * (f + 1)],
                            x16[:, o, 0:W],
                            start=(o == 0), stop=(o == DC - 1))
                    nc.scalar.activation(
                        out=h16[:, f, t0:t1], in_=ph[:], func=AF.Relu)
                ysb = hp.tile([P, DC, W], f32r, tag="yh", name=f"yh_{t0}")
                for m in range(DC):
                    py = ps_b.tile([P, W], f32, tag="b")
                    for f in range(FC):
                        nc.tensor.matmul(
                            py[:], sb_w2[:, f, P * m:P * (m + 1)],
                            h16[:, f, t0:t1],
                            start=(f == 0), stop=(f == FC - 1))
                    nc.vector.tensor_tensor(
                        out=ysb[:, m, 0:W], in0=py[:],
                        in1=xsb[:, m, 0:W], op=OP.add)
                _ln_cols(ysb, 7, 8, W, fast=fast)
                oap = rr(d_outs[sbase // 256].ap(), 256)
                for o in range(DC):   # per-chunk: store overlaps normalize
                    nc.sync.dma_start(oap[:, o, lc0:lc0 + W],
                                      ysb[:, o, 0:W])

            def attn_head(sl, h):
                pend = None
                for g in range(KC // 4):
                    new = attn_scores(sl, h, g)
                    if pend is not None:
                        attn_ctx(*pend)
                    pend = new
                attn_ctx(*pend)

            def attn_slice(sl):
                for h in range(H):
                    attn_head(sl, h)

            # ---- pipelined emission order ----------------------------
            for sl in range(3):
                t0, t1 = SLICES[sl]
                ctxh_t[t0] = hp.tile([P, DC, t1 - t0], fp8, tag="ctxh",
                                     name=f"ctxh_{t0}")
            attn_head(0, 0)
            attn_head(0, 1)
            qk_proj(1)
            attn_head(0, 2)
            attn_head(0, 3)
            qk_proj(2)
            attn_head(0, 4)
            attn_head(0, 5)
            qk_proj(3)
            attn_head(0, 6)
            attn_head(0, 7)
            attn_slice(1)
            x0 = tail_a(*SLICES[0])
            attn_slice(2)
            tail_b(*SLICES[0], *x0)
            x1 = tail_a(*SLICES[1])
            tail_b(*SLICES[1], *x1)
            x2 = tail_a(*SLICES[2])
            tail_b(*SLICES[2], *x2)

    nc.compile()
    return nc


def _get_program():
    if "nc" not in _CACHE:
        _CACHE["nc"] = _build_program()
    return _CACHE["nc"]


def _numpy_reference(src, recompute_idx, cached_idx, k_cached, v_cached,
                     in_proj_w, in_proj_b, out_proj_w, out_proj_b,
                     w1, b1, w2, b2, norm1_w, norm1_b, norm2_w, norm2_b):
    """Exact numpy translation of the oracle (general-case fallback)."""
    f = np.float32
    src = np.asarray(src, f)
    wq, wk, wv = in_proj_w[:D], in_proj_w[D:2 * D], in_proj_w[2 * D:]
    bq, bk, bv = in_proj_b[:D], in_proj_b[D:2 * D], in_proj_b[2 * D:]

    def ln(x, g, b):
        m = x.mean(-1, keepdims=True)
        v = x.var(-1, keepdims=True)
        return (x - m) / np.sqrt(v + EPS) * g + b

    q = (src @ wq.T + bq).reshape(B, S, H, HD).transpose(0, 2, 1, 3)
    src_rec = src[:, recompute_idx, :]
    k_rec = (src_rec @ wk.T + bk).reshape(B, -1, H, HD).transpose(0, 2, 1, 3)
    v_rec = (src_rec @ wv.T + bv).reshape(B, -1, H, HD).transpose(0, 2, 1, 3)
    k_full = np.zeros((B, H, S, HD), f)
    v_full = np.zeros((B, H, S, HD), f)
    k_full[:, :, cached_idx, :] = np.asarray(k_cached, f)[None]
    v_full[:, :, cached_idx, :] = np.asarray(v_cached, f)[None]
    k_full[:, :, recompute_idx, :] = k_rec
    v_full[:, :, recompute_idx, :] = v_rec
    scale = f(1.0 / np.sqrt(HD))
    scores = np.einsum("bhqd,bhkd->bhqk", q, k_full).astype(f) * scale
    scores -= scores.max(-1, keepdims=True)
    e = np.exp(scores)
    attn = e / e.sum(-1, keepdims=True)
    ctx = np.einsum("bhqk,bhkd->bhqd", attn, v_full).astype(f)
    ctx = ctx.transpose(0, 2, 1, 3).reshape(B, S, D)
    attn_out = ctx @ out_proj_w.T + out_proj_b
    x = ln(src + attn_out, norm1_w, norm1_b)
    ffn = np.maximum(x @ w1.T + b1, 0.0) @ w2.T + b2
    return ln(x + ffn, norm2_w, norm2_b).astype(f)


def _fp8(a):
    import ml_dtypes
    return np.ascontiguousarray(a).astype(ml_dtypes.float8_e4m3fn)


def _bf16(a):
    import ml_dtypes
    return np.ascontiguousarray(a).astype(ml_dtypes.bfloat16)


def _pmaj(x):
    """[n*P, cols] -> partition-major [P, n*cols] (contiguous)."""
    n = x.shape[0] // P
    return np.ascontiguousarray(
        x.reshape(n, P, x.shape[1]).transpose(1, 0, 2).reshape(P, -1))


def _pack_dr(wT, npair):
    """[npair*2*128, cols] -> DoubleRow layout [P, npair*2*cols]."""
    cols = wT.shape[1]
    return _fp8(wT.reshape(npair, 2, P, cols).transpose(2, 0, 1, 3)
                .reshape(P, -1))


def kernel(**inputs) -> np.ndarray:
    f = np.float32
    src = np.ascontiguousarray(np.asarray(inputs["src"], f))
    ridx = np.asarray(inputs["recompute_idx"]).astype(np.int64)
    cidx = np.asarray(inputs["cached_idx"]).astype(np.int64)

    # The fast path relies on {cached_idx} + {recompute_idx} being a
    # disjoint partition of [0, S), and is specialized to the problem
    # spec's zero FFN/out-proj/LN biases and unit LN gains (input_specs
    # fills).  Anything else -> exact numpy fallback.
    allidx = np.concatenate([ridx, cidx])
    if (len(ridx) != R or len(cidx) != SC
            or not np.array_equal(np.sort(allidx), np.arange(S))
            or np.any(np.asarray(inputs["out_proj_b"]))
            or np.any(np.asarray(inputs["b1"]))
            or np.any(np.asarray(inputs["b2"]))
            or np.any(np.asarray(inputs["norm1_b"]))
            or np.any(np.asarray(inputs["norm2_b"]))
            or not np.all(np.asarray(inputs["norm1_w"]) == 1.0)
            or not np.all(np.asarray(inputs["norm2_w"]) == 1.0)):
        return _numpy_reference(**inputs)

    in_proj_w = np.asarray(inputs["in_proj_w"], f)
    in_proj_b = np.asarray(inputs["in_proj_b"], f)
    out_proj_w = np.asarray(inputs["out_proj_w"], f)
    out_proj_b = np.asarray(inputs["out_proj_b"], f)
    w1 = np.asarray(inputs["w1"], f)
    b1 = np.asarray(inputs["b1"], f)
    w2 = np.asarray(inputs["w2"], f)
    b2 = np.asarray(inputs["b2"], f)
    k_cached = np.asarray(inputs["k_cached"], f)
    v_cached = np.asarray(inputs["v_cached"], f)

    wq, wk, wv = in_proj_w[:D], in_proj_w[D:2 * D], in_proj_w[2 * D:]
    bq, bk, bv = in_proj_b[:D], in_proj_b[D:2 * D], in_proj_b[2 * D:]

    wq8 = _pack_dr(np.ascontiguousarray(wq.T), 2)
    wk8 = _pack_dr(np.ascontiguousarray(wk.T), 2)
    wv8 = _pack_dr(np.ascontiguousarray(wv.T), 2)
    wo8 = _pack_dr(np.ascontiguousarray(out_proj_w.T), 2)
    w1P = _bf16(_pmaj(np.ascontiguousarray(w1.T)))
    w2P = _bf16(_pmaj(np.ascontiguousarray(w2.T)))
    vecsP = _pmaj(np.ascontiguousarray(np.stack(
        [bq, bk, bv, out_proj_b, b2,
         np.asarray(inputs["norm1_w"], f), np.asarray(inputs["norm1_b"], f),
         np.asarray(inputs["norm2_w"], f), np.asarray(inputs["norm2_b"], f)],
        axis=1)))
    bvrow = np.ascontiguousarray(np.tile(bv[None, :], (P, 1)))
    # packed cached-K: kc8[64*(h%2)+d, h//2, s] = k_cached[h, s, d]
    kct = k_cached.transpose(0, 2, 1)                  # [H, HD, SC]
    kc8 = _fp8(kct.reshape(4, 2, HD, SC).transpose(1, 2, 0, 3)
               .reshape(P, 4 * SC))
    # cached V in chunk-pair layout: ones column + zero pad to stride 80
    vca = np.concatenate(
        [v_cached.reshape(H, CC, P, HD), np.ones((H, CC, P, 1), f),
         np.zeros((H, CC, P, 80 - HD - 1), f)], axis=3)
    vv8 = _fp8(vca.transpose(2, 0, 1, 3).reshape(P, -1))

    shared = {
        "wq8P": wq8, "wk8P": wk8, "wv8P": wv8, "wo8P": wo8,
        "w1P": w1P, "w2P": w2P, "kc8P": kc8, "vv8P": vv8,
        "vecsP": vecsP, "bvrow": bvrow,
        "onesc": np.ones((P, 1), f),
        "b1c": np.ascontiguousarray(b1.reshape(FC, P).T),
    }
    srcR8 = [_fp8(_pmaj(np.ascontiguousarray(src[b][ridx].T)))
             for b in range(B)]

    in_maps = []
    for c in range(N_CORES):
        b, t = divmod(c, N_CORES // B)
        m = dict(shared)
        sP = _pmaj(np.ascontiguousarray(src[b, Q * t:Q * (t + 1), :].T))
        m["srcP"] = sP
        m["src8P"] = _fp8(sP)
        m["srcR8P"] = srcR8[b]
        in_maps.append(m)

    from concourse import bass_utils
    nc = _get_program()
    res = bass_utils.run_bass_kernel_spmd(
        nc, in_maps, core_ids=list(range(N_CORES)))

    out = np.empty((B, S, D), f)
    for c in range(N_CORES):
        b, t = divmod(c, N_CORES // B)
        outP = np.concatenate(
            [res.results[c][f"out{k}"].reshape(P, DC, 256)
             for k in range(3)], axis=2)        # [P, DC, Q]
        outT = outP.transpose(1, 0, 2).reshape(D, Q)
        out[b, Q * t:Q * (t + 1), :] = outT.T
    return out
